# revision 25
# baseline (speedup 1.0000x reference)
"""Causal GQA self-attention (B=2, S=2048, D=2048, 16 heads / 4 KV heads) on 8
Trainium2 NeuronCores.

Sharding: (batch, kv-head). Core c owns batch c//4 and KV head c%4, plus that
KV head's 4 query heads. Each core computes the full attention for its
(batch, kv-group) and a partial output projection over its heads' 512
y-dims; the host sums the 4 partial outputs per batch.

Device-side layout:
  - x is pre-transposed on host to xT [D, S_local]; the fused QKV projection
    runs in token-natural layout ([tok, 512 q | 128 k | 128 v]) contracting
    over D on partitions.
  - q/k are RMS-normed + RoPE'd in natural layout (free-dim math), rounded
    to bf16, then PE-transposed (bf16 identity, 1.0 cyc/row) into
    qT [hd, tok] / kT [hd, tok]. v stays natural in bf16.
  - The RMS-norm factor uses ACT Sqrt + DVE reciprocal (not Ln/Exp), so the
    scalar engine never swaps activation tables inside the QKV loop (the
    Exp table is loaded once for the attention phase).
  - Scores are computed TRANSPOSED: scT[k, q] = kT_tile^T @ qT (contraction
    over hd on partitions), so exp tiles feed the AV matmul directly with
    no per-tile PE transposes of the attention matrix.
  - Flash-style streaming: per (head, q-group of 512), k-tiles are
    processed in PAIRS sharing one 2-bank PSUM tile: 2 sc matmuls -> one
    ACT exp over [128,1024] -> bf16 [-> DVE causal mask mult on diagonal
    pairs] -> DVE pair-sum (halves the denominator matmul work) -> consume
    (lagging 2 pairs): {ones-matmul accumulating the softmax denominator in
    PSUM f32, 2 AV matmuls accumulating y}. Group finalization and the
    out-projection ride the same lag queue so the PE stream stays dense
    (holds the 2.4 GHz p-state).
  - 1/l is broadcast over partitions (Pool) and applied on AV evacuation.
"""

import math

import numpy as np

B = 2
S = 2048
D = 2048
T = B * S
NH = 16
NKV = 4
HD = 128
P = 128
ROPE_BASE = 10000.0
EPS = float(np.finfo(np.float32).eps)

N_CORES = 8
TT = S // P            # 16 token tiles per core (one batch)
GROUPS = 4             # q-groups of 512 queries
QKV = 768              # fused projection width: 4*q + k + v
SCALE = 1.0 / math.sqrt(HD)

_PROG = {}


def _build_program(loop_n=0):
    import concourse.mybir as mybir
    import concourse.tile as tile
    from concourse import bacc
    from concourse.masks import make_identity

    f32 = mybir.dt.float32
    f32r = mybir.dt.float32r
    bf16 = mybir.dt.bfloat16
    AL = mybir.AluOpType
    AF = mybir.ActivationFunctionType
    AX = mybir.AxisListType

    nc = bacc.Bacc("TRN2", target_bir_lowering=False, debug=False,
                   enable_asserts=True, num_devices=N_CORES)

    xT = nc.dram_tensor("xT", [D, S], f32r, kind="ExternalInput").ap()
    wcat = nc.dram_tensor("wcat", [D, QKV], f32r, kind="ExternalInput").ap()
    wpd = nc.dram_tensor("wpd", [4 * HD, D], bf16, kind="ExternalInput").ap()
    cosd = nc.dram_tensor("cosd", [S, HD // 2], f32, kind="ExternalInput").ap()
    sind = nc.dram_tensor("sind", [S, HD // 2], f32, kind="ExternalInput").ap()
    maskd = nc.dram_tensor("maskd", [P, 2, 1024], bf16, kind="ExternalInput").ap()
    gaind = nc.dram_tensor("gaind", [P, 6], f32, kind="ExternalInput").ap()
    outd = nc.dram_tensor("out", [S, D], f32, kind="ExternalOutput").ap()

    xT_r = xT.rearrange("(kt p) t -> p kt t", p=P)        # [128, 16, S]
    wcat_r = wcat.rearrange("(kt p) n -> p kt n", p=P)    # [128, 16, 768]
    wp_r = wpd.rearrange("(ct p) o -> p ct o", p=P)       # [128, 4, D]
    cos_r = cosd.rearrange("(tt p) f -> p tt f", p=P)     # [128, 16, 64]
    sin_r = sind.rearrange("(tt p) f -> p tt f", p=P)

    import contextlib as _ctxlib
    with tile.TileContext(nc) as tc, _ctxlib.ExitStack() as _es:
        pc = _es.enter_context(tc.tile_pool(name="const", bufs=1))
        pb = _es.enter_context(tc.tile_pool(name="batch", bufs=1))
        px = _es.enter_context(tc.tile_pool(name="xs", bufs=3))
        pstg = _es.enter_context(tc.tile_pool(name="stg", bufs=3))
        pscr = _es.enter_context(tc.tile_pool(name="scr", bufs=2))
        psml = _es.enter_context(tc.tile_pool(name="small", bufs=3))
        pqn = _es.enter_context(tc.tile_pool(name="qn", bufs=2))
        ptm = _es.enter_context(tc.tile_pool(name="ropetmp", bufs=2))
        prp = _es.enter_context(tc.tile_pool(name="rp", bufs=4))
        pep = _es.enter_context(tc.tile_pool(name="ep", bufs=4))
        pes = _es.enter_context(tc.tile_pool(name="es", bufs=3))
        pyT = _es.enter_context(tc.tile_pool(name="yT", bufs=2))
        prl = _es.enter_context(tc.tile_pool(name="rl", bufs=2))
        pob = _es.enter_context(tc.tile_pool(name="ob", bufs=2))
        ppC = _es.enter_context(tc.tile_pool(name="psC", bufs=2, space="PSUM"))
        ppT = _es.enter_context(tc.tile_pool(name="psT", bufs=1, space="PSUM"))
        ppA = _es.enter_context(tc.tile_pool(name="psA", bufs=2, space="PSUM"))
        ppL = _es.enter_context(tc.tile_pool(name="psL", bufs=1, space="PSUM"))

        # ---- constants resident in SBUF
        wcat_sb = pc.tile([P, TT, QKV], f32r, tag="wcat")
        for kt in range(TT):
            nc.sync.dma_start(wcat_sb[:, kt, :], wcat_r[:, kt, :])
        wp_sb = pc.tile([P, 4, D], bf16, tag="wp")
        for ct in range(4):
            nc.sync.dma_start(wp_sb[:, ct, :], wp_r[:, ct, :])
        cos_sb = pc.tile([P, TT, HD // 2], f32, tag="cos")
        nc.sync.dma_start(cos_sb[:], cos_r[:])
        sin_sb = pc.tile([P, TT, HD // 2], f32, tag="sin")
        nc.sync.dma_start(sin_sb[:], sin_r[:])
        mask_sb = pc.tile([P, 2, 1024], bf16, tag="mask")
        nc.sync.dma_start(mask_sb[:], maskd[:])
        gain_sb = pc.tile([P, 6], f32, tag="gain")
        nc.sync.dma_start(gain_sb[:], gaind[:])
        idf = pc.tile([P, P], f32, tag="idf")
        make_identity(nc, idf[:])
        idb = pc.tile([P, P], bf16, tag="idb")
        nc.vector.tensor_copy(idb[:], idf[:])
        ones_sb = pc.tile([P, 1], bf16, tag="ones")
        nc.vector.memset(ones_sb[:], 1.0)

        for _rep in range(max(1, loop_n)):
            qT = pb.tile([P, 4, S], bf16, tag="qT")       # [hd, h, tok]
            kT = pb.tile([P, S], bf16, tag="kT")          # [hd, tok]
            vN = pb.tile([P, TT, HD], bf16, tag="vN")     # [tok, tt, e]
            rps = {}

            # ---------------- QKV projection + RMS + RoPE ----------------
            def emit_tp(t):
                # transpose q0..q3,k of tile t into [hd, tok] and evac
                rp = rps.pop(t)
                tpb = ppT.tile([P, 640], bf16, tag="tpb", name="tpb")
                for s in range(5):
                    nc.tensor.transpose(tpb[:, s * P:(s + 1) * P],
                                        rp[:, s, :], idb[:])
                nc.vector.tensor_copy(
                    qT[:, :, t * P:(t + 1) * P],
                    tpb[:, :4 * P].rearrange("p (h x) -> p h x", h=4))
                nc.vector.tensor_copy(kT[:, t * P:(t + 1) * P],
                                      tpb[:, 4 * P:5 * P])

            xts = {}

            def fetch(t):
                if t < TT and t not in xts:
                    xtl = px.tile([P, TT, P], f32r, tag="xt")
                    nc.sync.dma_start(xtl[:], xT_r[:, :, t * P:(t + 1) * P])
                    xts[t] = xtl

            for tt in range(TT):
                fetch(tt)
                fetch(tt + 1)
                fetch(tt + 2)
                xt = xts.pop(tt)

                Ca = ppC.tile([P, 1024], f32, tag="C", name="Cqkv")
                for kt in range(TT):
                    nc.tensor.matmul(Ca[:, :512], xt[:, kt, :],
                                     wcat_sb[:, kt, :512],
                                     start=(kt == 0), stop=(kt == TT - 1))
                for kt in range(TT):
                    nc.tensor.matmul(Ca[:, 512:768], xt[:, kt, :],
                                     wcat_sb[:, kt, 512:768],
                                     start=(kt == 0), stop=(kt == TT - 1))

                # stage q0..q3,k in SBUF f32; v straight to bf16
                stg = pstg.tile([P, 5, P], f32, tag="stg")
                nc.scalar.copy(stg[:].rearrange("p s x -> p (s x)"),
                               Ca[:, :640])
                nc.scalar.copy(vN[:, tt, :], Ca[:, 640:768])

                # rms-norm factors: rs = exp(-.5*ln(ssq/HD+eps)) * gain
                scr = pscr.tile([P, 5, P], f32, tag="scr")
                nc.vector.tensor_tensor(scr[:], stg[:], stg[:], AL.mult)
                ssq = psml.tile([P, 5], f32, tag="ssq")
                nc.vector.tensor_reduce(ssq[:], scr[:], axis=AX.X, op=AL.add)
                # rs = 1/sqrt(ssq/HD + eps): ACT Sqrt (stays off the Exp
                # table set) + DVE reciprocal
                sq5 = psml.tile([P, 5], f32, tag="sq5")
                nc.scalar.activation(sq5[:], ssq[:], AF.Sqrt,
                                     scale=1.0 / HD, bias=gain_sb[:, 5:6])
                rs5 = psml.tile([P, 5], f32, tag="rs5")
                nc.vector.reciprocal(rs5[:], sq5[:])
                rsg = psml.tile([P, 5], f32, tag="rsg")
                nc.vector.tensor_tensor(rsg[:], rs5[:], gain_sb[:, :5], AL.mult)

                qn = pqn.tile([P, 5, P], f32, tag="qn")
                nc.vector.tensor_tensor(
                    qn[:], stg[:],
                    rsg[:, :, None].to_broadcast([P, 5, P]), AL.mult)

                # rope: o1 = a*cos + b*sin ; o2 = b*cos - a*sin  (bf16 out)
                a = qn[:, :, :HD // 2]
                b2 = qn[:, :, HD // 2:]
                cb = cos_sb[:, None, tt, :].to_broadcast([P, 5, HD // 2])
                sb_ = sin_sb[:, None, tt, :].to_broadcast([P, 5, HD // 2])
                rp = prp.tile([P, 5, P], bf16, tag="rp")
                rps[tt] = rp
                t1 = ptm.tile([P, 5, HD // 2], f32, tag="t1")
                t2 = ptm.tile([P, 5, HD // 2], f32, tag="t2")
                nc.gpsimd.tensor_tensor(t1[:], a, cb, AL.mult)
                nc.vector.tensor_tensor(t2[:], b2, sb_, AL.mult)
                nc.vector.tensor_tensor(rp[:, :, :HD // 2], t1[:], t2[:], AL.add)
                t3 = ptm.tile([P, 5, HD // 2], f32, tag="t3")
                t4 = ptm.tile([P, 5, HD // 2], f32, tag="t4")
                nc.gpsimd.tensor_tensor(t3[:], b2, cb, AL.mult)
                nc.vector.tensor_tensor(t4[:], a, sb_, AL.mult)
                nc.vector.tensor_tensor(rp[:, :, HD // 2:], t3[:], t4[:],
                                        AL.subtract)

                if tt >= 2:
                    emit_tp(tt - 2)

            # ---------------- attention + interleaved out-proj ------------
            # Flat emission stream: sc matmul+exp per k-tile; consumes
            # (ones-mm + AV) lag 3 tiles behind; group finalization and
            # out-proj ride the same queue so the PE never waits.
            import collections
            pending = collections.deque()

            def fin(h, ya, ls, yt):
                def run():
                    rl = prl.tile([1, 512], f32, tag="rl")
                    nc.vector.reciprocal(rl[:], ls[:])
                    rlb = prl.tile([P, 512], f32, tag="rlb")
                    nc.gpsimd.partition_broadcast(rlb[:], rl[0:1, :])
                    nc.vector.tensor_tensor(yt[:, h, :], ya[:], rlb[:], AL.mult)
                return run

            def oproj(g, yt):
                def run():
                    for tl in range(4):
                        r0 = g * 512 + tl * P
                        ob = pob.tile([P, D], f32, tag="ob")
                        for half in range(2):
                            Cp = ppC.tile([P, 1024], f32, tag="C", name="Cpr")
                            for oc in range(2):
                                c0 = half * 1024 + oc * 512
                                for ct in range(4):
                                    nc.tensor.matmul(
                                        Cp[:, oc * 512:(oc + 1) * 512],
                                        yt[:, ct, tl * P:(tl + 1) * P],
                                        wp_sb[:, ct, c0:c0 + 512],
                                        start=(ct == 0), stop=(ct == 3))
                            dst = ob[:, half * 1024:(half + 1) * 1024]
                            if half == 0:
                                nc.scalar.copy(dst, Cp[:])
                            else:
                                nc.vector.tensor_copy(dst, Cp[:])
                        nc.sync.dma_start(outd[r0:r0 + P, :], ob[:])
                return run

            def pump(target):
                while len(pending) > target:
                    pending.popleft()()

            for g in range(GROUPS):
                nj = 4 * (g + 1)
                npair = nj // 2
                yt = pyT.tile([P, 4, 512], bf16, tag="yt", name=f"yt{g}")
                for h in range(4):
                    ya = ppA.tile([P, 512], f32, tag="ya", name="ya")
                    ls = ppL.tile([1, 512], f32, tag="ls", name="ls")
                    qs = qT[:, h, g * 512:(g + 1) * 512]
                    for p in range(npair):
                        sc = ppC.tile([P, 1024], f32, tag="C", name="sc")
                        nc.tensor.matmul(sc[:, :512],
                                         kT[:, 2 * p * P:(2 * p + 1) * P], qs,
                                         start=True, stop=True)
                        nc.tensor.matmul(sc[:, 512:],
                                         kT[:, (2 * p + 1) * P:(2 * p + 2) * P],
                                         qs, start=True, stop=True)
                        ep = pep.tile([P, 1024], bf16, tag="ep")
                        nc.scalar.activation(ep[:], sc[:], AF.Exp)
                        if p >= 2 * g:
                            nc.vector.tensor_tensor(
                                ep[:], ep[:], mask_sb[:, p - 2 * g, :],
                                AL.mult)
                        es = pes.tile([P, 512], bf16, tag="es")
                        nc.vector.tensor_tensor(es[:], ep[:, :512],
                                                ep[:, 512:], AL.add)

                        def consume(ep=ep, es=es, p=p, ya=ya, ls=ls,
                                    nj=nj, npair=npair):
                            nc.tensor.matmul(ls[:], ones_sb[:], es[:],
                                             start=(p == 0),
                                             stop=(p == npair - 1))
                            nc.tensor.matmul(ya[:], vN[:, 2 * p, :],
                                             ep[:, :512],
                                             start=(p == 0), stop=False)
                            nc.tensor.matmul(ya[:], vN[:, 2 * p + 1, :],
                                             ep[:, 512:],
                                             start=False,
                                             stop=(p == npair - 1))
                        pending.append(consume)
                        pump(2)
                    pending.append(fin(h, ya, ls, yt))
                    if g == 0 and h in (0, 1):
                        # late qk transposes, covered by attention work
                        pending.append(lambda t=14 + h: emit_tp(t))
                pending.append(oproj(g, yt))
            pump(0)

    nc.compile()
    return nc


def _get_program(loop_n=0):
    key = loop_n
    if key not in _PROG:
        _PROG[key] = _build_program(loop_n)
    return _PROG[key]


def _host_prep(x, Wq, Wk, Wv, Wp, q_gain):
    """Build the 8 per-core input maps. Core c = (batch c//4, kv head c%4)."""
    import ml_dtypes
    bf16 = ml_dtypes.bfloat16

    inv_freq = 1.0 / (ROPE_BASE ** (np.arange(0, HD, 2, dtype=np.float32) / HD))
    freqs = np.arange(S, dtype=np.float32)[:, None] * inv_freq[None, :]
    cos = np.ascontiguousarray(np.cos(freqs).astype(np.float32))
    sin = np.ascontiguousarray(np.sin(freqs).astype(np.float32))

    # causal 0/1 masks for the diagonal-chunk tile variants (il = 0..3),
    # packed as pairs: variant v holds [il=2v | il=2v+1] side by side.
    # tile rows k (128), group columns q (512): valid iff q >= il*128 + k
    k = np.arange(P)[:, None, None]
    il = np.arange(4)[None, :, None]
    q = np.arange(512)[None, None, :]
    masks = (q >= il * P + k).astype(bf16)               # [128, 4, 512]
    masks = masks.reshape(P, 2, 1024)                    # [128, 2, 1024]

    in_maps = []
    for core in range(N_CORES):
        b, kv = divmod(core, 4)
        h0 = 4 * kv
        xT = np.ascontiguousarray(
            x[b].reshape(S, D).T.astype(np.float32))     # [D, S]
        WqT = Wq[h0 * HD:(h0 + 4) * HD, :].T             # [D, 512]
        WkT = Wk[kv * HD:(kv + 1) * HD, :].T             # [D, 128]
        WvT = Wv[kv * HD:(kv + 1) * HD, :].T             # [D, 128]
        wcat = np.ascontiguousarray(
            np.concatenate([WqT, WkT, WvT], axis=1), dtype=np.float32)
        wpT = np.ascontiguousarray(
            Wp[:, h0 * HD:(h0 + 4) * HD].T.astype(bf16))  # [512, D]
        gain = np.tile(np.array(
            [[q_gain[h0] * SCALE, q_gain[h0 + 1] * SCALE,
              q_gain[h0 + 2] * SCALE, q_gain[h0 + 3] * SCALE,
              1.0, EPS]], dtype=np.float32), (P, 1))
        in_maps.append({
            "xT": xT,
            "wcat": wcat,
            "wpd": wpT,
            "cosd": cos,
            "sind": sin,
            "maskd": np.ascontiguousarray(masks),
            "gaind": np.ascontiguousarray(gain),
        })
    return in_maps


def kernel(x, Wq, Wk, Wv, Wp, q_gain):
    from concourse.bass_utils import run_bass_kernel_spmd

    nc = _get_program()
    in_maps = _host_prep(x, Wq, Wk, Wv, Wp, q_gain)
    try:
        res = run_bass_kernel_spmd(nc, in_maps, core_ids=list(range(N_CORES)))
    except Exception:
        # one retry: a previous crashed run can leave the exec unit wedged
        res = run_bass_kernel_spmd(nc, in_maps, core_ids=list(range(N_CORES)))
    out = np.zeros((B, S, D), dtype=np.float32)
    for core in range(N_CORES):
        out[core // 4] += res.results[core]["out"]
    return out


# revision 48
# speedup vs baseline: 2.3116x; 2.3116x over previous
"""Causal GQA self-attention (B=2, S=2048, D=2048, 16 heads / 4 KV heads) on 8
Trainium2 NeuronCores.

Sharding: (batch, kv-head). Core c owns batch c//4 and KV head c%4, plus that
KV head's 4 query heads. Each core computes the full attention for its
(batch, kv-group) and a partial output projection over its heads' 512
y-dims; the host sums the 4 partial outputs per batch.

Device-side layout:
  - x is pre-transposed on host to xT [D, S_local]; the fused QKV projection
    runs in token-natural layout ([tok, 512 q | 128 k | 128 v]) contracting
    over D on partitions.
  - q/k are RMS-normed + RoPE'd in natural layout (free-dim math), rounded
    to bf16, then PE-transposed (bf16 identity, 1.0 cyc/row) into
    qT [hd, tok] / kT [hd, tok]. v stays natural in bf16.
  - The RMS-norm factor uses ACT Sqrt + DVE reciprocal (not Ln/Exp), so the
    scalar engine never swaps activation tables inside the QKV loop (the
    Exp table is loaded once for the attention phase).
  - Scores are computed TRANSPOSED: scT[k, q] = kT_tile^T @ qT (contraction
    over hd on partitions), so exp tiles feed the AV matmul directly with
    no per-tile PE transposes of the attention matrix.
  - Flash-style streaming: per (head, q-group of 512), k-tiles are
    processed in PAIRS sharing one 2-bank PSUM tile: 2 sc matmuls -> one
    ACT exp over [128,1024] -> bf16 [-> DVE causal mask mult on diagonal
    pairs] -> DVE pair-sum (halves the denominator matmul work) -> consume
    (lagging 2 pairs): {ones-matmul accumulating the softmax denominator in
    PSUM f32, 2 AV matmuls accumulating y}. Group finalization and the
    out-projection ride the same lag queue so the PE stream stays dense
    (holds the 2.4 GHz p-state).
  - 1/l is broadcast over partitions (Pool) and applied on AV evacuation.
"""

import math

import numpy as np

B = 2
S = 2048
D = 2048
T = B * S
NH = 16
NKV = 4
HD = 128
P = 128
ROPE_BASE = 10000.0
EPS = float(np.finfo(np.float32).eps)

N_CORES = 8
TT = S // P            # 16 token tiles per core (one batch)
GROUPS = 4             # q-groups of 512 queries
QKV = 768              # fused projection width: 4*q + k + v
SCALE = 1.0 / math.sqrt(HD)

_PROG = {}


def _build_program(loop_n=0):
    import concourse.mybir as mybir
    import concourse.tile as tile
    from concourse import bacc
    from concourse.masks import make_identity

    f32 = mybir.dt.float32
    f32r = mybir.dt.float32r
    bf16 = mybir.dt.bfloat16
    AL = mybir.AluOpType
    AF = mybir.ActivationFunctionType
    AX = mybir.AxisListType

    nc = bacc.Bacc("TRN2", target_bir_lowering=False, debug=False,
                   enable_asserts=True, num_devices=N_CORES)

    xT = nc.dram_tensor("xT", [D, S], bf16, kind="ExternalInput").ap()
    wcat = nc.dram_tensor("wcat", [D, QKV], bf16, kind="ExternalInput").ap()
    wpd = nc.dram_tensor("wpd", [4 * HD, D], bf16, kind="ExternalInput").ap()
    cosd = nc.dram_tensor("cosd", [S, HD // 2], f32, kind="ExternalInput").ap()
    sind = nc.dram_tensor("sind", [S, HD // 2], f32, kind="ExternalInput").ap()
    maskd = nc.dram_tensor("maskd", [P, 2, 1024], bf16, kind="ExternalInput").ap()
    gaind = nc.dram_tensor("gaind", [P, 6], f32, kind="ExternalInput").ap()
    outd = nc.dram_tensor("out", [S, D], f32, kind="ExternalOutput").ap()

    xT_r = xT.rearrange("(kt p) t -> p kt t", p=P)        # [128, 16, S]
    wcat_r = wcat.rearrange("(kt p) n -> p kt n", p=P)    # [128, 16, 768]
    wp_r = wpd.rearrange("(ct p) o -> p ct o", p=P)       # [128, 4, D]
    cos_r = cosd.rearrange("(tt p) f -> p tt f", p=P)     # [128, 16, 64]
    sin_r = sind.rearrange("(tt p) f -> p tt f", p=P)

    import contextlib as _ctxlib
    with tile.TileContext(nc) as tc, _ctxlib.ExitStack() as _es:
        pc = _es.enter_context(tc.tile_pool(name="const", bufs=1))
        pb = _es.enter_context(tc.tile_pool(name="batch", bufs=1))
        px = _es.enter_context(tc.tile_pool(name="xs", bufs=3))
        pstg = _es.enter_context(tc.tile_pool(name="stg", bufs=3))
        pscr = _es.enter_context(tc.tile_pool(name="scr", bufs=2))
        psml = _es.enter_context(tc.tile_pool(name="small", bufs=3))
        pqn = _es.enter_context(tc.tile_pool(name="qn", bufs=2))
        ptm = _es.enter_context(tc.tile_pool(name="ropetmp", bufs=2))
        prp = _es.enter_context(tc.tile_pool(name="rp", bufs=4))
        pep = _es.enter_context(tc.tile_pool(name="ep", bufs=4))
        pes = _es.enter_context(tc.tile_pool(name="es", bufs=4))
        pe2 = _es.enter_context(tc.tile_pool(name="es2", bufs=2))
        pyT = _es.enter_context(tc.tile_pool(name="yT", bufs=2))
        prl = _es.enter_context(tc.tile_pool(name="rl", bufs=2))
        pob = _es.enter_context(tc.tile_pool(name="ob", bufs=2))
        ppC = _es.enter_context(tc.tile_pool(name="psC", bufs=2, space="PSUM"))
        ppT = _es.enter_context(tc.tile_pool(name="psT", bufs=1, space="PSUM"))
        ppA = _es.enter_context(tc.tile_pool(name="psA", bufs=2, space="PSUM"))
        ppL = _es.enter_context(tc.tile_pool(name="psL", bufs=1, space="PSUM"))

        # ---- constants resident in SBUF
        wcat_sb = pc.tile([P, TT, QKV], bf16, tag="wcat")
        for kt in range(TT):
            nc.sync.dma_start(wcat_sb[:, kt, :], wcat_r[:, kt, :])
        wp_sb = pc.tile([P, 4, D], bf16, tag="wp")
        for ct in range(4):
            nc.sync.dma_start(wp_sb[:, ct, :], wp_r[:, ct, :])
        cos_sb = pc.tile([P, TT, HD // 2], f32, tag="cos")
        nc.sync.dma_start(cos_sb[:], cos_r[:])
        sin_sb = pc.tile([P, TT, HD // 2], f32, tag="sin")
        nc.sync.dma_start(sin_sb[:], sin_r[:])
        mask_sb = pc.tile([P, 2, 1024], bf16, tag="mask")
        nc.sync.dma_start(mask_sb[:], maskd[:])
        gain_sb = pc.tile([P, 6], f32, tag="gain")
        nc.sync.dma_start(gain_sb[:], gaind[:])
        idf = pc.tile([P, P], f32, tag="idf")
        make_identity(nc, idf[:])
        idb = pc.tile([P, P], bf16, tag="idb")
        nc.vector.tensor_copy(idb[:], idf[:])
        ones_sb = pc.tile([P, 1], bf16, tag="ones")
        nc.vector.memset(ones_sb[:], 1.0)

        import collections
        fillers = collections.deque()

        for _rep in range(max(1, loop_n)):
            qT = pb.tile([P, 4, S], bf16, tag="qT")       # [hd, h, tok]
            kT = pb.tile([P, S], bf16, tag="kT")          # [hd, tok]
            vN = pb.tile([P, TT, HD], bf16, tag="vN")     # [tok, tt, e]
            rps = {}

            # ---------------- QKV projection + RMS + RoPE ----------------
            def emit_tp(t):
                # transpose q0..q3,k of tile t into [hd, tok] and evac
                rp = rps.pop(t)
                tpb = ppT.tile([P, 640], bf16, tag="tpb", name="tpb")
                for s in range(5):
                    nc.tensor.transpose(tpb[:, s * P:(s + 1) * P],
                                        rp[:, s, :], idb[:])
                nc.scalar.copy(
                    qT[:, :, t * P:(t + 1) * P],
                    tpb[:, :4 * P].rearrange("p (h x) -> p h x", h=4))
                nc.scalar.copy(kT[:, t * P:(t + 1) * P],
                               tpb[:, 4 * P:5 * P])

            xts = {}

            def fetch(t):
                if t < TT and t not in xts:
                    xtl = px.tile([P, TT, P], bf16, tag="xt")
                    nc.sync.dma_start(xtl[:], xT_r[:, :, t * P:(t + 1) * P])
                    xts[t] = xtl

            for tt in range(TT):
                fetch(tt)
                fetch(tt + 1)
                fetch(tt + 2)
                xt = xts.pop(tt)
                # drain leftover out-proj work from the previous rep into
                # this rep's projection stream
                for _ in range(2):
                    if fillers:
                        fillers.popleft()()

                Ca = ppC.tile([P, 1024], f32, tag="C", name="Cqkv")
                for kt in range(TT):
                    nc.tensor.matmul(Ca[:, :512], xt[:, kt, :],
                                     wcat_sb[:, kt, :512],
                                     start=(kt == 0), stop=(kt == TT - 1))
                for kt in range(TT):
                    nc.tensor.matmul(Ca[:, 512:768], xt[:, kt, :],
                                     wcat_sb[:, kt, 512:768],
                                     start=(kt == 0), stop=(kt == TT - 1))

                # stage q0..q3,k in SBUF f32; v straight to bf16
                stg = pstg.tile([P, 5, P], f32, tag="stg")
                nc.scalar.copy(stg[:].rearrange("p s x -> p (s x)"),
                               Ca[:, :640])
                nc.scalar.copy(vN[:, tt, :], Ca[:, 640:768])

                # rms-norm factors: rs = exp(-.5*ln(ssq/HD+eps)) * gain
                scr = pscr.tile([P, 5, P], f32, tag="scr")
                nc.vector.tensor_tensor(scr[:], stg[:], stg[:], AL.mult)
                ssq = psml.tile([P, 5], f32, tag="ssq")
                nc.vector.tensor_reduce(ssq[:], scr[:], axis=AX.X, op=AL.add)
                # rs = 1/sqrt(ssq/HD + eps): ACT Sqrt (stays off the Exp
                # table set) + DVE reciprocal
                sq5 = psml.tile([P, 5], f32, tag="sq5")
                nc.scalar.activation(sq5[:], ssq[:], AF.Sqrt,
                                     scale=1.0 / HD, bias=gain_sb[:, 5:6])
                rs5 = psml.tile([P, 5], f32, tag="rs5")
                nc.vector.reciprocal(rs5[:], sq5[:])
                rsg = psml.tile([P, 5], f32, tag="rsg")
                nc.vector.tensor_tensor(rsg[:], rs5[:], gain_sb[:, :5], AL.mult)

                qn = pqn.tile([P, 5, P], f32, tag="qn")
                nc.vector.tensor_tensor(
                    qn[:], stg[:],
                    rsg[:, :, None].to_broadcast([P, 5, P]), AL.mult)

                # rope: o1 = a*cos + b*sin ; o2 = b*cos - a*sin  (bf16 out)
                a = qn[:, :, :HD // 2]
                b2 = qn[:, :, HD // 2:]
                cb = cos_sb[:, None, tt, :].to_broadcast([P, 5, HD // 2])
                sb_ = sin_sb[:, None, tt, :].to_broadcast([P, 5, HD // 2])
                rp = prp.tile([P, 5, P], bf16, tag="rp")
                rps[tt] = rp
                t1 = ptm.tile([P, 5, HD // 2], f32, tag="t1")
                t2 = ptm.tile([P, 5, HD // 2], f32, tag="t2")
                nc.gpsimd.tensor_tensor(t1[:], a, cb, AL.mult)
                nc.vector.tensor_tensor(t2[:], b2, sb_, AL.mult)
                nc.vector.tensor_tensor(rp[:, :, :HD // 2], t1[:], t2[:], AL.add)
                t3 = ptm.tile([P, 5, HD // 2], f32, tag="t3")
                t4 = ptm.tile([P, 5, HD // 2], f32, tag="t4")
                nc.gpsimd.tensor_tensor(t3[:], b2, cb, AL.mult)
                nc.vector.tensor_tensor(t4[:], a, sb_, AL.mult)
                nc.vector.tensor_tensor(rp[:, :, HD // 2:], t3[:], t4[:],
                                        AL.subtract)

                if tt >= 2:
                    emit_tp(tt - 2)

            # ---------------- attention + interleaved out-proj ------------
            # Flat emission stream: sc matmul+exp per k-tile; consumes
            # (ones-mm + AV) lag 3 tiles behind; group finalization and
            # out-proj ride the same queue so the PE never waits.
            import collections
            pending = collections.deque()

            def fin(h, ya, ls, yt):
                def run():
                    rl = prl.tile([1, 512], f32, tag="rl")
                    nc.vector.reciprocal(rl[:], ls[:])
                    rlb = prl.tile([P, 512], f32, tag="rlb")
                    nc.gpsimd.partition_broadcast(rlb[:], rl[0:1, :])
                    nc.vector.tensor_tensor(yt[:, h, :], ya[:], rlb[:], AL.mult)
                return run

            def oproj(g, yt):
                # out-proj emitted as fine-grained filler items so the PE
                # interleaves them with the next group's attention stream
                # (keeps PE per-slot work ahead of ACT's exp rate).
                items = []
                state = {}
                for tl in range(4):
                    for half in range(2):
                        def blk(tl=tl, half=half):
                            Cp = ppC.tile([P, 1024], f32, tag="C", name="Cpr")
                            state[(tl, half)] = Cp
                            for oc in range(2):
                                c0 = half * 1024 + oc * 512
                                for ct in range(4):
                                    nc.tensor.matmul(
                                        Cp[:, oc * 512:(oc + 1) * 512],
                                        yt[:, ct, tl * P:(tl + 1) * P],
                                        wp_sb[:, ct, c0:c0 + 512],
                                        start=(ct == 0), stop=(ct == 3))
                        def evac(tl=tl, half=half):
                            Cp = state.pop((tl, half))
                            if half == 0:
                                ob = pob.tile([P, D], f32, tag="ob")
                                state[tl] = ob
                                nc.scalar.copy(ob[:, :1024], Cp[:])
                            else:
                                ob = state.pop(tl)
                                nc.vector.tensor_copy(ob[:, 1024:], Cp[:])
                                r0 = g * 512 + tl * P
                                nc.sync.dma_start(outd[r0:r0 + P, :], ob[:])
                        items.append(blk)
                        items.append(evac)
                return items

            def pump(target):
                while len(pending) > target:
                    pending.popleft()()
                    if fillers:
                        fillers.popleft()()

            for g in range(GROUPS):
                nj = 4 * (g + 1)
                npair = nj // 2
                yt = pyT.tile([P, 4, 512], bf16, tag="yt", name=f"yt{g}")
                for h in range(4):
                    ya = ppA.tile([P, 512], f32, tag="ya", name="ya")
                    ls = ppL.tile([1, 512], f32, tag="ls", name="ls")
                    qs = qT[:, h, g * 512:(g + 1) * 512]
                    ess = {}
                    for p in range(npair):
                        sc = ppC.tile([P, 1024], f32, tag="C", name="sc")
                        nc.tensor.matmul(sc[:, :512],
                                         kT[:, 2 * p * P:(2 * p + 1) * P], qs,
                                         start=True, stop=True)
                        nc.tensor.matmul(sc[:, 512:],
                                         kT[:, (2 * p + 1) * P:(2 * p + 2) * P],
                                         qs, start=True, stop=True)
                        ep = pep.tile([P, 1024], bf16, tag="ep")
                        nc.scalar.activation(ep[:], sc[:], AF.Exp)
                        if p >= 2 * g:
                            nc.vector.tensor_tensor(
                                ep[:], ep[:], mask_sb[:, p - 2 * g, :],
                                AL.mult)
                        es = pes.tile([P, 512], bf16, tag="es")
                        nc.vector.tensor_tensor(es[:], ep[:, :512],
                                                ep[:, 512:], AL.add)
                        e2 = None
                        if p % 2 == 1:
                            # fold two pair-sums eagerly: one PE reduction
                            # per 4 k-tiles
                            e2 = pe2.tile([P, 512], bf16, tag="es2")
                            nc.vector.tensor_tensor(
                                e2[:], ess.pop(p - 1)[:], es[:], AL.add)
                        else:
                            ess[p] = es

                        def consume(ep=ep, e2=e2, p=p, ya=ya, ls=ls,
                                    nj=nj, npair=npair):
                            if e2 is not None:
                                nc.tensor.matmul(ls[:], ones_sb[:], e2[:],
                                                 start=(p == 1),
                                                 stop=(p == npair - 1))
                            nc.tensor.matmul(ya[:], vN[:, 2 * p, :],
                                             ep[:, :512],
                                             start=(p == 0), stop=False)
                            nc.tensor.matmul(ya[:], vN[:, 2 * p + 1, :],
                                             ep[:, 512:],
                                             start=False,
                                             stop=(p == npair - 1))
                        pending.append(consume)
                        pump(2)
                    pending.append(fin(h, ya, ls, yt))
                    if g == 1 and h in (0, 1):
                        # late qk transposes, covered by attention work
                        pending.append(lambda t=14 + h: emit_tp(t))
                items = oproj(g, yt)
                pending.append(lambda items=items: fillers.extend(items))
            pump(0)

        while fillers:
            fillers.popleft()()

    nc.compile()
    return nc


def _get_program(loop_n=0):
    key = loop_n
    if key not in _PROG:
        _PROG[key] = _build_program(loop_n)
    return _PROG[key]


def _host_prep(x, Wq, Wk, Wv, Wp, q_gain):
    """Build the 8 per-core input maps. Core c = (batch c//4, kv head c%4)."""
    import ml_dtypes
    bf16 = ml_dtypes.bfloat16

    inv_freq = 1.0 / (ROPE_BASE ** (np.arange(0, HD, 2, dtype=np.float32) / HD))
    freqs = np.arange(S, dtype=np.float32)[:, None] * inv_freq[None, :]
    cos = np.ascontiguousarray(np.cos(freqs).astype(np.float32))
    sin = np.ascontiguousarray(np.sin(freqs).astype(np.float32))

    # causal 0/1 masks for the diagonal-chunk tile variants (il = 0..3),
    # packed as pairs: variant v holds [il=2v | il=2v+1] side by side.
    # tile rows k (128), group columns q (512): valid iff q >= il*128 + k
    k = np.arange(P)[:, None, None]
    il = np.arange(4)[None, :, None]
    q = np.arange(512)[None, None, :]
    masks = (q >= il * P + k).astype(bf16)               # [128, 4, 512]
    masks = masks.reshape(P, 2, 1024)                    # [128, 2, 1024]

    in_maps = []
    for core in range(N_CORES):
        b, kv = divmod(core, 4)
        h0 = 4 * kv
        xT = np.ascontiguousarray(
            x[b].reshape(S, D).T.astype(bf16))           # [D, S]
        WqT = Wq[h0 * HD:(h0 + 4) * HD, :].T             # [D, 512]
        WkT = Wk[kv * HD:(kv + 1) * HD, :].T             # [D, 128]
        WvT = Wv[kv * HD:(kv + 1) * HD, :].T             # [D, 128]
        wcat = np.ascontiguousarray(
            np.concatenate([WqT, WkT, WvT], axis=1).astype(bf16))
        wpT = np.ascontiguousarray(
            Wp[:, h0 * HD:(h0 + 4) * HD].T.astype(bf16))  # [512, D]
        gain = np.tile(np.array(
            [[q_gain[h0] * SCALE, q_gain[h0 + 1] * SCALE,
              q_gain[h0 + 2] * SCALE, q_gain[h0 + 3] * SCALE,
              1.0, EPS]], dtype=np.float32), (P, 1))
        in_maps.append({
            "xT": xT,
            "wcat": wcat,
            "wpd": wpT,
            "cosd": cos,
            "sind": sin,
            "maskd": np.ascontiguousarray(masks),
            "gaind": np.ascontiguousarray(gain),
        })
    return in_maps


def kernel(x, Wq, Wk, Wv, Wp, q_gain):
    from concourse.bass_utils import run_bass_kernel_spmd

    nc = _get_program()
    in_maps = _host_prep(x, Wq, Wk, Wv, Wp, q_gain)
    try:
        res = run_bass_kernel_spmd(nc, in_maps, core_ids=list(range(N_CORES)))
    except Exception:
        # one retry: a previous crashed run can leave the exec unit wedged
        res = run_bass_kernel_spmd(nc, in_maps, core_ids=list(range(N_CORES)))
    out = np.zeros((B, S, D), dtype=np.float32)
    for core in range(N_CORES):
        out[core // 4] += res.results[core]["out"]
    return out


# revision 56
# speedup vs baseline: 2.3429x; 1.0135x over previous
"""Causal GQA self-attention (B=2, S=2048, D=2048, 16 heads / 4 KV heads) on 8
Trainium2 NeuronCores.

Sharding: (batch, kv-head). Core c owns batch c//4 and KV head c%4, plus that
KV head's 4 query heads. Each core computes the full attention for its
(batch, kv-group) and a partial output projection over its heads' 512
y-dims; the host sums the 4 partial outputs per batch.

Device-side layout:
  - x is pre-transposed on host to xT [D, S_local]; the fused QKV projection
    runs in token-natural layout ([tok, 512 q | 128 k | 128 v]) contracting
    over D on partitions.
  - q/k are RMS-normed + RoPE'd in natural layout (free-dim math), rounded
    to bf16, then PE-transposed (bf16 identity, 1.0 cyc/row) into
    qT [hd, tok] / kT [hd, tok]. v stays natural in bf16.
  - The RMS-norm factor uses ACT Sqrt + DVE reciprocal (not Ln/Exp), so the
    scalar engine never swaps activation tables inside the QKV loop (the
    Exp table is loaded once for the attention phase).
  - Scores are computed TRANSPOSED: scT[k, q] = kT_tile^T @ qT (contraction
    over hd on partitions), so exp tiles feed the AV matmul directly with
    no per-tile PE transposes of the attention matrix.
  - Flash-style streaming: per (head, q-group of 512), k-tiles are
    processed in PAIRS sharing one 2-bank PSUM tile: 2 sc matmuls -> one
    ACT exp over [128,1024] -> bf16 [-> DVE causal mask mult on diagonal
    pairs] -> DVE pair-sum (halves the denominator matmul work) -> consume
    (lagging 2 pairs): {ones-matmul accumulating the softmax denominator in
    PSUM f32, 2 AV matmuls accumulating y}. Group finalization and the
    out-projection ride the same lag queue so the PE stream stays dense
    (holds the 2.4 GHz p-state).
  - 1/l is broadcast over partitions (Pool) and applied on AV evacuation.
"""

import math

import numpy as np

B = 2
S = 2048
D = 2048
T = B * S
NH = 16
NKV = 4
HD = 128
P = 128
ROPE_BASE = 10000.0
EPS = float(np.finfo(np.float32).eps)

N_CORES = 8
TT = S // P            # 16 token tiles per core (one batch)
GROUPS = 4             # q-groups of 512 queries
QKV = 768              # fused projection width: 4*q + k + v
SCALE = 1.0 / math.sqrt(HD)

_PROG = {}


def _build_program(loop_n=0):
    import concourse.mybir as mybir
    import concourse.tile as tile
    from concourse import bacc
    from concourse.masks import make_identity

    f32 = mybir.dt.float32
    f32r = mybir.dt.float32r
    bf16 = mybir.dt.bfloat16
    AL = mybir.AluOpType
    AF = mybir.ActivationFunctionType
    AX = mybir.AxisListType

    nc = bacc.Bacc("TRN2", target_bir_lowering=False, debug=False,
                   enable_asserts=True, num_devices=N_CORES)

    xT = nc.dram_tensor("xT", [D, S], bf16, kind="ExternalInput").ap()
    wcat = nc.dram_tensor("wcat", [D, QKV], bf16, kind="ExternalInput").ap()
    wpd = nc.dram_tensor("wpd", [4 * HD, D], bf16, kind="ExternalInput").ap()
    cosd = nc.dram_tensor("cosd", [S, HD // 2], f32, kind="ExternalInput").ap()
    sind = nc.dram_tensor("sind", [S, HD // 2], f32, kind="ExternalInput").ap()
    maskd = nc.dram_tensor("maskd", [P, 2, 1024], bf16, kind="ExternalInput").ap()
    gaind = nc.dram_tensor("gaind", [P, 6], f32, kind="ExternalInput").ap()
    outd = nc.dram_tensor("out", [S, D], f32, kind="ExternalOutput").ap()

    xT_r = xT.rearrange("(kt p) t -> p kt t", p=P)        # [128, 16, S]
    wcat_r = wcat.rearrange("(kt p) n -> p kt n", p=P)    # [128, 16, 768]
    wp_r = wpd.rearrange("(ct p) o -> p ct o", p=P)       # [128, 4, D]
    cos_r = cosd.rearrange("(tt p) f -> p tt f", p=P)     # [128, 16, 64]
    sin_r = sind.rearrange("(tt p) f -> p tt f", p=P)

    import contextlib as _ctxlib
    with tile.TileContext(nc) as tc, _ctxlib.ExitStack() as _es:
        pc = _es.enter_context(tc.tile_pool(name="const", bufs=1))
        pb = _es.enter_context(tc.tile_pool(name="batch", bufs=1))
        px = _es.enter_context(tc.tile_pool(name="xs", bufs=3))
        pstg = _es.enter_context(tc.tile_pool(name="stg", bufs=3))
        pscr = _es.enter_context(tc.tile_pool(name="scr", bufs=2))
        psml = _es.enter_context(tc.tile_pool(name="small", bufs=3))
        pqn = _es.enter_context(tc.tile_pool(name="qn", bufs=2))
        ptm = _es.enter_context(tc.tile_pool(name="ropetmp", bufs=2))
        prp = _es.enter_context(tc.tile_pool(name="rp", bufs=4))
        pep = _es.enter_context(tc.tile_pool(name="ep", bufs=6))
        pes = _es.enter_context(tc.tile_pool(name="es", bufs=6))
        pe2 = _es.enter_context(tc.tile_pool(name="es2", bufs=3))
        pyT = _es.enter_context(tc.tile_pool(name="yT", bufs=2))
        prl = _es.enter_context(tc.tile_pool(name="rl", bufs=2))
        pob = _es.enter_context(tc.tile_pool(name="ob", bufs=2))
        ppC = _es.enter_context(tc.tile_pool(name="psC", bufs=2, space="PSUM"))
        ppT = _es.enter_context(tc.tile_pool(name="psT", bufs=1, space="PSUM"))
        ppA = _es.enter_context(tc.tile_pool(name="psA", bufs=2, space="PSUM"))
        ppL = _es.enter_context(tc.tile_pool(name="psL", bufs=1, space="PSUM"))

        # ---- constants resident in SBUF
        wcat_sb = pc.tile([P, TT, QKV], bf16, tag="wcat")
        for kt in range(TT):
            nc.sync.dma_start(wcat_sb[:, kt, :], wcat_r[:, kt, :])
        wp_sb = pc.tile([P, 4, D], bf16, tag="wp")
        for ct in range(4):
            nc.sync.dma_start(wp_sb[:, ct, :], wp_r[:, ct, :])
        cos_sb = pc.tile([P, TT, HD // 2], f32, tag="cos")
        nc.sync.dma_start(cos_sb[:], cos_r[:])
        sin_sb = pc.tile([P, TT, HD // 2], f32, tag="sin")
        nc.sync.dma_start(sin_sb[:], sin_r[:])
        mask_sb = pc.tile([P, 2, 1024], bf16, tag="mask")
        nc.sync.dma_start(mask_sb[:], maskd[:])
        gain_sb = pc.tile([P, 6], f32, tag="gain")
        nc.sync.dma_start(gain_sb[:], gaind[:])
        idf = pc.tile([P, P], f32, tag="idf")
        make_identity(nc, idf[:])
        idb = pc.tile([P, P], bf16, tag="idb")
        nc.vector.tensor_copy(idb[:], idf[:])
        ones_sb = pc.tile([P, 1], bf16, tag="ones")
        nc.vector.memset(ones_sb[:], 1.0)

        import collections
        fillers = collections.deque()

        for _rep in range(max(1, loop_n)):
            qT = pb.tile([P, 4, S], bf16, tag="qT")       # [hd, h, tok]
            kT = pb.tile([P, S], bf16, tag="kT")          # [hd, tok]
            vN = pb.tile([P, TT, HD], bf16, tag="vN")     # [tok, tt, e]
            rps = {}

            # ---------------- QKV projection + RMS + RoPE ----------------
            def emit_tp(t):
                # transpose q0..q3,k of tile t into [hd, tok] and evac
                rp = rps.pop(t)
                tpb = ppT.tile([P, 640], bf16, tag="tpb", name="tpb")
                for s in range(5):
                    nc.tensor.transpose(tpb[:, s * P:(s + 1) * P],
                                        rp[:, s, :], idb[:])
                nc.scalar.copy(
                    qT[:, :, t * P:(t + 1) * P],
                    tpb[:, :4 * P].rearrange("p (h x) -> p h x", h=4))
                nc.scalar.copy(kT[:, t * P:(t + 1) * P],
                               tpb[:, 4 * P:5 * P])

            xts = {}

            def fetch(t):
                if t < TT and t not in xts:
                    xtl = px.tile([P, TT, P], bf16, tag="xt")
                    nc.sync.dma_start(xtl[:], xT_r[:, :, t * P:(t + 1) * P])
                    xts[t] = xtl

            for tt in range(TT):
                fetch(tt)
                fetch(tt + 1)
                fetch(tt + 2)
                xt = xts.pop(tt)
                # drain leftover out-proj work from the previous rep into
                # this rep's projection stream
                for _ in range(2):
                    if fillers:
                        fillers.popleft()()

                Ca = ppC.tile([P, 1024], f32, tag="C", name="Cqkv")
                for kt in range(TT):
                    nc.tensor.matmul(Ca[:, :512], xt[:, kt, :],
                                     wcat_sb[:, kt, :512],
                                     start=(kt == 0), stop=(kt == TT - 1))
                for kt in range(TT):
                    nc.tensor.matmul(Ca[:, 512:768], xt[:, kt, :],
                                     wcat_sb[:, kt, 512:768],
                                     start=(kt == 0), stop=(kt == TT - 1))

                # stage q0..q3,k in SBUF f32; v straight to bf16
                stg = pstg.tile([P, 5, P], f32, tag="stg")
                nc.scalar.copy(stg[:].rearrange("p s x -> p (s x)"),
                               Ca[:, :640])
                nc.scalar.copy(vN[:, tt, :], Ca[:, 640:768])

                # rms-norm factors: rs = exp(-.5*ln(ssq/HD+eps)) * gain
                scr = pscr.tile([P, 5, P], f32, tag="scr")
                nc.vector.tensor_tensor(scr[:], stg[:], stg[:], AL.mult)
                ssq = psml.tile([P, 5], f32, tag="ssq")
                nc.vector.tensor_reduce(ssq[:], scr[:], axis=AX.X, op=AL.add)
                # rs = 1/sqrt(ssq/HD + eps): ACT Sqrt (stays off the Exp
                # table set) + DVE reciprocal
                sq5 = psml.tile([P, 5], f32, tag="sq5")
                nc.scalar.activation(sq5[:], ssq[:], AF.Sqrt,
                                     scale=1.0 / HD, bias=gain_sb[:, 5:6])
                rs5 = psml.tile([P, 5], f32, tag="rs5")
                nc.vector.reciprocal(rs5[:], sq5[:])
                rsg = psml.tile([P, 5], f32, tag="rsg")
                nc.vector.tensor_tensor(rsg[:], rs5[:], gain_sb[:, :5], AL.mult)

                qn = pqn.tile([P, 5, P], f32, tag="qn")
                nc.vector.tensor_tensor(
                    qn[:], stg[:],
                    rsg[:, :, None].to_broadcast([P, 5, P]), AL.mult)

                # rope: o1 = a*cos + b*sin ; o2 = b*cos - a*sin  (bf16 out)
                a = qn[:, :, :HD // 2]
                b2 = qn[:, :, HD // 2:]
                cb = cos_sb[:, None, tt, :].to_broadcast([P, 5, HD // 2])
                sb_ = sin_sb[:, None, tt, :].to_broadcast([P, 5, HD // 2])
                rp = prp.tile([P, 5, P], bf16, tag="rp")
                rps[tt] = rp
                t1 = ptm.tile([P, 5, HD // 2], f32, tag="t1")
                t2 = ptm.tile([P, 5, HD // 2], f32, tag="t2")
                nc.gpsimd.tensor_tensor(t1[:], a, cb, AL.mult)
                nc.vector.tensor_tensor(t2[:], b2, sb_, AL.mult)
                nc.vector.tensor_tensor(rp[:, :, :HD // 2], t1[:], t2[:], AL.add)
                t3 = ptm.tile([P, 5, HD // 2], f32, tag="t3")
                t4 = ptm.tile([P, 5, HD // 2], f32, tag="t4")
                nc.gpsimd.tensor_tensor(t3[:], b2, cb, AL.mult)
                nc.vector.tensor_tensor(t4[:], a, sb_, AL.mult)
                nc.vector.tensor_tensor(rp[:, :, HD // 2:], t3[:], t4[:],
                                        AL.subtract)

                if tt >= 2:
                    emit_tp(tt - 2)

            # ---------------- attention + interleaved out-proj ------------
            # Flat emission stream: sc matmul+exp per k-tile; consumes
            # (ones-mm + AV) lag 3 tiles behind; group finalization and
            # out-proj ride the same queue so the PE never waits.
            import collections
            pending = collections.deque()

            def fin(h, ya, ls, yt):
                def run():
                    rl = prl.tile([1, 512], f32, tag="rl")
                    nc.vector.reciprocal(rl[:], ls[:])
                    rlb = prl.tile([P, 512], f32, tag="rlb")
                    nc.gpsimd.partition_broadcast(rlb[:], rl[0:1, :])
                    nc.vector.tensor_tensor(yt[:, h, :], ya[:], rlb[:], AL.mult)
                return run

            def oproj(g, yt):
                # out-proj emitted as fine-grained filler items so the PE
                # interleaves them with the next group's attention stream
                # (keeps PE per-slot work ahead of ACT's exp rate).
                items = []
                state = {}
                for tl in range(4):
                    for half in range(2):
                        def blk(tl=tl, half=half):
                            Cp = ppC.tile([P, 1024], f32, tag="C", name="Cpr")
                            state[(tl, half)] = Cp
                            for oc in range(2):
                                c0 = half * 1024 + oc * 512
                                for ct in range(4):
                                    nc.tensor.matmul(
                                        Cp[:, oc * 512:(oc + 1) * 512],
                                        yt[:, ct, tl * P:(tl + 1) * P],
                                        wp_sb[:, ct, c0:c0 + 512],
                                        start=(ct == 0), stop=(ct == 3))
                        def evac(tl=tl, half=half):
                            Cp = state.pop((tl, half))
                            if half == 0:
                                ob = pob.tile([P, D], f32, tag="ob")
                                state[tl] = ob
                                nc.scalar.copy(ob[:, :1024], Cp[:])
                            else:
                                ob = state.pop(tl)
                                nc.vector.tensor_copy(ob[:, 1024:], Cp[:])
                                r0 = g * 512 + tl * P
                                nc.sync.dma_start(outd[r0:r0 + P, :], ob[:])
                        items.append(blk)
                        items.append(evac)
                return items

            def pump(target):
                while len(pending) > target:
                    pending.popleft()()
                    if fillers:
                        fillers.popleft()()

            for g in range(GROUPS):
                nj = 4 * (g + 1)
                npair = nj // 2
                yt = pyT.tile([P, 4, 512], bf16, tag="yt", name=f"yt{g}")
                for h in range(4):
                    ya = ppA.tile([P, 512], f32, tag="ya", name="ya")
                    ls = ppL.tile([1, 512], f32, tag="ls", name="ls")
                    qs = qT[:, h, g * 512:(g + 1) * 512]
                    ess = {}
                    for p in range(npair):
                        # diagonal pairs: skip score columns that are fully
                        # above the causal diagonal (exp of the stale psum
                        # there is finite and zeroed by the mask)
                        d = p - 2 * g
                        c0 = max(0, 2 * d) * P
                        c1 = max(0, 2 * d + 1) * P
                        sc = ppC.tile([P, 1024], f32, tag="C", name="sc")
                        nc.tensor.matmul(sc[:, c0:512],
                                         kT[:, 2 * p * P:(2 * p + 1) * P],
                                         qs[:, c0:], start=True, stop=True)
                        nc.tensor.matmul(sc[:, 512 + c1:],
                                         kT[:, (2 * p + 1) * P:(2 * p + 2) * P],
                                         qs[:, c1:], start=True, stop=True)
                        ep = pep.tile([P, 1024], bf16, tag="ep")
                        nc.scalar.activation(ep[:], sc[:], AF.Exp)
                        if p >= 2 * g:
                            nc.vector.tensor_tensor(
                                ep[:], ep[:], mask_sb[:, p - 2 * g, :],
                                AL.mult)
                        es = pes.tile([P, 512], bf16, tag="es")
                        nc.vector.tensor_tensor(es[:], ep[:, :512],
                                                ep[:, 512:], AL.add)
                        e2 = None
                        if p % 2 == 1:
                            # fold two pair-sums eagerly: one PE reduction
                            # per 4 k-tiles
                            e2 = pe2.tile([P, 512], bf16, tag="es2")
                            nc.vector.tensor_tensor(
                                e2[:], ess.pop(p - 1)[:], es[:], AL.add)
                        else:
                            ess[p] = es

                        def consume(ep=ep, e2=e2, p=p, g=g, ya=ya, ls=ls,
                                    nj=nj, npair=npair):
                            if e2 is not None:
                                nc.tensor.matmul(ls[:], ones_sb[:], e2[:],
                                                 start=(p == 1),
                                                 stop=(p == npair - 1))
                            for half in range(2):
                                jt = 2 * p + half
                                eph = ep[:, half * 512:(half + 1) * 512]
                                dt = jt - 4 * g
                                if dt < 0:
                                    nc.tensor.matmul(ya[:], vN[:, jt, :],
                                                     eph,
                                                     start=(jt == 0),
                                                     stop=False)
                                else:
                                    # column block dt takes its last
                                    # contribution here; later blocks go on
                                    b0, b1 = dt * P, (dt + 1) * P
                                    nc.tensor.matmul(ya[:, b0:b1],
                                                     vN[:, jt, :],
                                                     eph[:, b0:b1],
                                                     start=(jt == 0),
                                                     stop=True)
                                    if dt < 3:
                                        nc.tensor.matmul(ya[:, b1:512],
                                                         vN[:, jt, :],
                                                         eph[:, b1:512],
                                                         start=(jt == 0),
                                                         stop=False)
                        pending.append(consume)
                        pump(4)
                    pending.append(fin(h, ya, ls, yt))
                    if g == 1 and h in (0, 1):
                        # late qk transposes, covered by attention work
                        pending.append(lambda t=14 + h: emit_tp(t))
                items = oproj(g, yt)
                pending.append(lambda items=items: fillers.extend(items))
            pump(0)

        while fillers:
            fillers.popleft()()

    nc.compile()
    return nc


def _get_program(loop_n=0):
    key = loop_n
    if key not in _PROG:
        _PROG[key] = _build_program(loop_n)
    return _PROG[key]


def _host_prep(x, Wq, Wk, Wv, Wp, q_gain):
    """Build the 8 per-core input maps. Core c = (batch c//4, kv head c%4)."""
    import ml_dtypes
    bf16 = ml_dtypes.bfloat16

    inv_freq = 1.0 / (ROPE_BASE ** (np.arange(0, HD, 2, dtype=np.float32) / HD))
    freqs = np.arange(S, dtype=np.float32)[:, None] * inv_freq[None, :]
    cos = np.ascontiguousarray(np.cos(freqs).astype(np.float32))
    sin = np.ascontiguousarray(np.sin(freqs).astype(np.float32))

    # causal 0/1 masks for the diagonal-chunk tile variants (il = 0..3),
    # packed as pairs: variant v holds [il=2v | il=2v+1] side by side.
    # tile rows k (128), group columns q (512): valid iff q >= il*128 + k
    k = np.arange(P)[:, None, None]
    il = np.arange(4)[None, :, None]
    q = np.arange(512)[None, None, :]
    masks = (q >= il * P + k).astype(bf16)               # [128, 4, 512]
    masks = masks.reshape(P, 2, 1024)                    # [128, 2, 1024]

    in_maps = []
    for core in range(N_CORES):
        b, kv = divmod(core, 4)
        h0 = 4 * kv
        xT = np.ascontiguousarray(
            x[b].reshape(S, D).T.astype(bf16))           # [D, S]
        WqT = Wq[h0 * HD:(h0 + 4) * HD, :].T             # [D, 512]
        WkT = Wk[kv * HD:(kv + 1) * HD, :].T             # [D, 128]
        WvT = Wv[kv * HD:(kv + 1) * HD, :].T             # [D, 128]
        wcat = np.ascontiguousarray(
            np.concatenate([WqT, WkT, WvT], axis=1).astype(bf16))
        wpT = np.ascontiguousarray(
            Wp[:, h0 * HD:(h0 + 4) * HD].T.astype(bf16))  # [512, D]
        gain = np.tile(np.array(
            [[q_gain[h0] * SCALE, q_gain[h0 + 1] * SCALE,
              q_gain[h0 + 2] * SCALE, q_gain[h0 + 3] * SCALE,
              1.0, EPS]], dtype=np.float32), (P, 1))
        in_maps.append({
            "xT": xT,
            "wcat": wcat,
            "wpd": wpT,
            "cosd": cos,
            "sind": sin,
            "maskd": np.ascontiguousarray(masks),
            "gaind": np.ascontiguousarray(gain),
        })
    return in_maps


def kernel(x, Wq, Wk, Wv, Wp, q_gain):
    from concourse.bass_utils import run_bass_kernel_spmd

    nc = _get_program()
    in_maps = _host_prep(x, Wq, Wk, Wv, Wp, q_gain)
    try:
        res = run_bass_kernel_spmd(nc, in_maps, core_ids=list(range(N_CORES)))
    except Exception:
        # one retry: a previous crashed run can leave the exec unit wedged
        res = run_bass_kernel_spmd(nc, in_maps, core_ids=list(range(N_CORES)))
    out = np.zeros((B, S, D), dtype=np.float32)
    for core in range(N_CORES):
        out[core // 4] += res.results[core]["out"]
    return out


# revision 58
# speedup vs baseline: 2.7117x; 1.1574x over previous
"""Causal GQA self-attention (B=2, S=2048, D=2048, 16 heads / 4 KV heads) on 8
Trainium2 NeuronCores.

Sharding: (batch, kv-head). Core c owns batch c//4 and KV head c%4, plus that
KV head's 4 query heads. Each core computes the full attention for its
(batch, kv-group) and a partial output projection over its heads' 512
y-dims; the host sums the 4 partial outputs per batch.

Device-side layout:
  - x is pre-transposed on host to xT [D, S_local]; the fused QKV projection
    runs in token-natural layout ([tok, 512 q | 128 k | 128 v]) contracting
    over D on partitions.
  - q/k are RMS-normed + RoPE'd in natural layout (free-dim math), rounded
    to bf16, then PE-transposed (bf16 identity, 1.0 cyc/row) into
    qT [hd, tok] / kT [hd, tok]. v stays natural in bf16.
  - The RMS-norm factor uses ACT Sqrt + DVE reciprocal (not Ln/Exp), so the
    scalar engine never swaps activation tables inside the QKV loop (the
    Exp table is loaded once for the attention phase).
  - Scores are computed TRANSPOSED: scT[k, q] = kT_tile^T @ qT (contraction
    over hd on partitions), so exp tiles feed the AV matmul directly with
    no per-tile PE transposes of the attention matrix.
  - Flash-style streaming: per (head, q-group of 512), k-tiles are
    processed in PAIRS sharing one 2-bank PSUM tile: 2 sc matmuls (partial
    width on diagonal pairs) -> one ACT exp over [128,1024] -> bf16
    [-> DVE causal mask mult on diagonal pairs] -> DVE pair-sums folded
    into f32 partial denominators -> consume (lagging 4 pairs): AV matmuls
    with per-column-block stop flags (each 128-col block stops at its
    diagonal k-tile). The softmax denominator never touches the PE: partial
    sums are folded on DVE and partition-all-reduced on Pool, freeing PSUM
    banks for a 3-deep score pipeline.
  - Group finalization and the out-projection ride lag/filler queues (the
    out-proj also spills across rep boundaries) so the PE stream stays
    dense and holds the 2.4 GHz p-state.
  - 1/l (reciprocal of the all-reduced sums) is applied on AV evacuation.
"""

import math

import numpy as np

B = 2
S = 2048
D = 2048
T = B * S
NH = 16
NKV = 4
HD = 128
P = 128
ROPE_BASE = 10000.0
EPS = float(np.finfo(np.float32).eps)

N_CORES = 8
TT = S // P            # 16 token tiles per core (one batch)
GROUPS = 4             # q-groups of 512 queries
QKV = 768              # fused projection width: 4*q + k + v
SCALE = 1.0 / math.sqrt(HD)

_PROG = {}


def _build_program(loop_n=0):
    import concourse.mybir as mybir
    import concourse.tile as tile
    from concourse import bacc
    from concourse.masks import make_identity
    from concourse import bass_isa

    f32 = mybir.dt.float32
    f32r = mybir.dt.float32r
    bf16 = mybir.dt.bfloat16
    AL = mybir.AluOpType
    AF = mybir.ActivationFunctionType
    AX = mybir.AxisListType

    nc = bacc.Bacc("TRN2", target_bir_lowering=False, debug=False,
                   enable_asserts=True, num_devices=N_CORES)

    xT = nc.dram_tensor("xT", [D, S], bf16, kind="ExternalInput").ap()
    wcat = nc.dram_tensor("wcat", [D, QKV], bf16, kind="ExternalInput").ap()
    wpd = nc.dram_tensor("wpd", [4 * HD, D], bf16, kind="ExternalInput").ap()
    cosd = nc.dram_tensor("cosd", [S, HD // 2], f32, kind="ExternalInput").ap()
    sind = nc.dram_tensor("sind", [S, HD // 2], f32, kind="ExternalInput").ap()
    maskd = nc.dram_tensor("maskd", [P, 2, 1024], bf16, kind="ExternalInput").ap()
    gaind = nc.dram_tensor("gaind", [P, 6], f32, kind="ExternalInput").ap()
    outd = nc.dram_tensor("out", [S, D], f32, kind="ExternalOutput").ap()

    xT_r = xT.rearrange("(kt p) t -> p kt t", p=P)        # [128, 16, S]
    wcat_r = wcat.rearrange("(kt p) n -> p kt n", p=P)    # [128, 16, 768]
    wp_r = wpd.rearrange("(ct p) o -> p ct o", p=P)       # [128, 4, D]
    cos_r = cosd.rearrange("(tt p) f -> p tt f", p=P)     # [128, 16, 64]
    sin_r = sind.rearrange("(tt p) f -> p tt f", p=P)

    import contextlib as _ctxlib
    with tile.TileContext(nc) as tc, _ctxlib.ExitStack() as _es:
        pc = _es.enter_context(tc.tile_pool(name="const", bufs=1))
        pb = _es.enter_context(tc.tile_pool(name="batch", bufs=1))
        px = _es.enter_context(tc.tile_pool(name="xs", bufs=3))
        pstg = _es.enter_context(tc.tile_pool(name="stg", bufs=3))
        pscr = _es.enter_context(tc.tile_pool(name="scr", bufs=2))
        psml = _es.enter_context(tc.tile_pool(name="small", bufs=3))
        pqn = _es.enter_context(tc.tile_pool(name="qn", bufs=2))
        ptm = _es.enter_context(tc.tile_pool(name="ropetmp", bufs=2))
        prp = _es.enter_context(tc.tile_pool(name="rp", bufs=4))
        pep = _es.enter_context(tc.tile_pool(name="ep", bufs=6))
        pes = _es.enter_context(tc.tile_pool(name="es", bufs=6))
        pe2 = _es.enter_context(tc.tile_pool(name="es2", bufs=3))
        pyT = _es.enter_context(tc.tile_pool(name="yT", bufs=2))
        prl = _es.enter_context(tc.tile_pool(name="rl", bufs=2))
        pob = _es.enter_context(tc.tile_pool(name="ob", bufs=2))
        pacc = _es.enter_context(tc.tile_pool(name="acc", bufs=3))
        ppC = _es.enter_context(tc.tile_pool(name="psC", bufs=3, space="PSUM"))
        ppT = _es.enter_context(tc.tile_pool(name="psT", bufs=1, space="PSUM"))
        ppA = _es.enter_context(tc.tile_pool(name="psA", bufs=1, space="PSUM"))

        # ---- constants resident in SBUF
        wcat_sb = pc.tile([P, TT, QKV], bf16, tag="wcat")
        for kt in range(TT):
            nc.sync.dma_start(wcat_sb[:, kt, :], wcat_r[:, kt, :])
        wp_sb = pc.tile([P, 4, D], bf16, tag="wp")
        for ct in range(4):
            nc.sync.dma_start(wp_sb[:, ct, :], wp_r[:, ct, :])
        cos_sb = pc.tile([P, TT, HD // 2], f32, tag="cos")
        nc.sync.dma_start(cos_sb[:], cos_r[:])
        sin_sb = pc.tile([P, TT, HD // 2], f32, tag="sin")
        nc.sync.dma_start(sin_sb[:], sin_r[:])
        mask_sb = pc.tile([P, 2, 1024], bf16, tag="mask")
        nc.sync.dma_start(mask_sb[:], maskd[:])
        gain_sb = pc.tile([P, 6], f32, tag="gain")
        nc.sync.dma_start(gain_sb[:], gaind[:])
        idf = pc.tile([P, P], f32, tag="idf")
        make_identity(nc, idf[:])
        idb = pc.tile([P, P], bf16, tag="idb")
        nc.vector.tensor_copy(idb[:], idf[:])
        import collections
        fillers = collections.deque()

        for _rep in range(max(1, loop_n)):
            qT = pb.tile([P, 4, S], bf16, tag="qT")       # [hd, h, tok]
            kT = pb.tile([P, S], bf16, tag="kT")          # [hd, tok]
            vN = pb.tile([P, TT, HD], bf16, tag="vN")     # [tok, tt, e]
            rps = {}

            # ---------------- QKV projection + RMS + RoPE ----------------
            def emit_tp(t):
                # transpose q0..q3,k of tile t into [hd, tok] and evac
                rp = rps.pop(t)
                tpb = ppT.tile([P, 640], bf16, tag="tpb", name="tpb")
                for s in range(5):
                    nc.tensor.transpose(tpb[:, s * P:(s + 1) * P],
                                        rp[:, s, :], idb[:])
                nc.scalar.copy(
                    qT[:, :, t * P:(t + 1) * P],
                    tpb[:, :4 * P].rearrange("p (h x) -> p h x", h=4))
                nc.scalar.copy(kT[:, t * P:(t + 1) * P],
                               tpb[:, 4 * P:5 * P])

            xts = {}

            def fetch(t):
                if t < TT and t not in xts:
                    xtl = px.tile([P, TT, P], bf16, tag="xt")
                    nc.sync.dma_start(xtl[:], xT_r[:, :, t * P:(t + 1) * P])
                    xts[t] = xtl

            for tt in range(TT):
                fetch(tt)
                fetch(tt + 1)
                fetch(tt + 2)
                xt = xts.pop(tt)
                # drain leftover out-proj work from the previous rep into
                # this rep's projection stream
                for _ in range(2):
                    if fillers:
                        fillers.popleft()()

                Ca = ppC.tile([P, 1024], f32, tag="C", name="Cqkv")
                for kt in range(TT):
                    nc.tensor.matmul(Ca[:, :512], xt[:, kt, :],
                                     wcat_sb[:, kt, :512],
                                     start=(kt == 0), stop=(kt == TT - 1))
                for kt in range(TT):
                    nc.tensor.matmul(Ca[:, 512:768], xt[:, kt, :],
                                     wcat_sb[:, kt, 512:768],
                                     start=(kt == 0), stop=(kt == TT - 1))

                # stage q0..q3,k in SBUF f32; v straight to bf16
                stg = pstg.tile([P, 5, P], f32, tag="stg")
                nc.scalar.copy(stg[:].rearrange("p s x -> p (s x)"),
                               Ca[:, :640])
                nc.scalar.copy(vN[:, tt, :], Ca[:, 640:768])

                # rms-norm factors: rs = exp(-.5*ln(ssq/HD+eps)) * gain
                scr = pscr.tile([P, 5, P], f32, tag="scr")
                nc.vector.tensor_tensor(scr[:], stg[:], stg[:], AL.mult)
                ssq = psml.tile([P, 5], f32, tag="ssq")
                nc.vector.tensor_reduce(ssq[:], scr[:], axis=AX.X, op=AL.add)
                # rs = 1/sqrt(ssq/HD + eps): ACT Sqrt (stays off the Exp
                # table set) + DVE reciprocal
                sq5 = psml.tile([P, 5], f32, tag="sq5")
                nc.scalar.activation(sq5[:], ssq[:], AF.Sqrt,
                                     scale=1.0 / HD, bias=gain_sb[:, 5:6])
                rs5 = psml.tile([P, 5], f32, tag="rs5")
                nc.vector.reciprocal(rs5[:], sq5[:])
                rsg = psml.tile([P, 5], f32, tag="rsg")
                nc.vector.tensor_tensor(rsg[:], rs5[:], gain_sb[:, :5], AL.mult)

                qn = pqn.tile([P, 5, P], f32, tag="qn")
                nc.vector.tensor_tensor(
                    qn[:], stg[:],
                    rsg[:, :, None].to_broadcast([P, 5, P]), AL.mult)

                # rope: o1 = a*cos + b*sin ; o2 = b*cos - a*sin  (bf16 out)
                a = qn[:, :, :HD // 2]
                b2 = qn[:, :, HD // 2:]
                cb = cos_sb[:, None, tt, :].to_broadcast([P, 5, HD // 2])
                sb_ = sin_sb[:, None, tt, :].to_broadcast([P, 5, HD // 2])
                rp = prp.tile([P, 5, P], bf16, tag="rp")
                rps[tt] = rp
                t1 = ptm.tile([P, 5, HD // 2], f32, tag="t1")
                t2 = ptm.tile([P, 5, HD // 2], f32, tag="t2")
                nc.gpsimd.tensor_tensor(t1[:], a, cb, AL.mult)
                nc.vector.tensor_tensor(t2[:], b2, sb_, AL.mult)
                nc.vector.tensor_tensor(rp[:, :, :HD // 2], t1[:], t2[:], AL.add)
                t3 = ptm.tile([P, 5, HD // 2], f32, tag="t3")
                t4 = ptm.tile([P, 5, HD // 2], f32, tag="t4")
                nc.gpsimd.tensor_tensor(t3[:], b2, cb, AL.mult)
                nc.vector.tensor_tensor(t4[:], a, sb_, AL.mult)
                nc.vector.tensor_tensor(rp[:, :, HD // 2:], t3[:], t4[:],
                                        AL.subtract)

                if tt >= 2:
                    emit_tp(tt - 2)

            # ---------------- attention + interleaved out-proj ------------
            # Flat emission stream: sc matmul+exp per k-tile; consumes
            # (ones-mm + AV) lag 3 tiles behind; group finalization and
            # out-proj ride the same queue so the PE never waits.
            import collections
            pending = collections.deque()

            def fin(h, ya, acc, yt):
                def run():
                    # softmax denominators: all-reduce the key-partition
                    # partial sums (f32 internally), then reciprocal
                    rls = prl.tile([P, 512], f32, tag="rls")
                    nc.gpsimd.partition_all_reduce(
                        rls[:], acc[:], channels=P,
                        reduce_op=bass_isa.ReduceOp.add)
                    rlb = prl.tile([P, 512], f32, tag="rlb")
                    nc.vector.reciprocal(rlb[:], rls[:])
                    nc.vector.tensor_tensor(yt[:, h, :], ya[:], rlb[:], AL.mult)
                return run

            def oproj(g, yt):
                # out-proj emitted as fine-grained filler items so the PE
                # interleaves them with the next group's attention stream
                # (keeps PE per-slot work ahead of ACT's exp rate).
                items = []
                state = {}
                for tl in range(4):
                    for half in range(2):
                        def blk(tl=tl, half=half):
                            Cp = ppC.tile([P, 1024], f32, tag="C", name="Cpr")
                            state[(tl, half)] = Cp
                            for oc in range(2):
                                c0 = half * 1024 + oc * 512
                                for ct in range(4):
                                    nc.tensor.matmul(
                                        Cp[:, oc * 512:(oc + 1) * 512],
                                        yt[:, ct, tl * P:(tl + 1) * P],
                                        wp_sb[:, ct, c0:c0 + 512],
                                        start=(ct == 0), stop=(ct == 3))
                        def evac(tl=tl, half=half):
                            Cp = state.pop((tl, half))
                            if half == 0:
                                ob = pob.tile([P, D], f32, tag="ob")
                                state[tl] = ob
                                nc.scalar.copy(ob[:, :1024], Cp[:])
                            else:
                                ob = state.pop(tl)
                                nc.vector.tensor_copy(ob[:, 1024:], Cp[:])
                                r0 = g * 512 + tl * P
                                nc.sync.dma_start(outd[r0:r0 + P, :], ob[:])
                        items.append(blk)
                        items.append(evac)
                return items

            def pump(target):
                while len(pending) > target:
                    pending.popleft()()
                    if fillers:
                        fillers.popleft()()

            for g in range(GROUPS):
                nj = 4 * (g + 1)
                npair = nj // 2
                yt = pyT.tile([P, 4, 512], bf16, tag="yt", name=f"yt{g}")
                for h in range(4):
                    ya = ppA.tile([P, 512], f32, tag="ya", name="ya")
                    qs = qT[:, h, g * 512:(g + 1) * 512]
                    ess = {}
                    acc = None
                    for p in range(npair):
                        # diagonal pairs: skip score columns that are fully
                        # above the causal diagonal (exp of the stale psum
                        # there is finite and zeroed by the mask)
                        d = p - 2 * g
                        c0 = max(0, 2 * d) * P
                        c1 = max(0, 2 * d + 1) * P
                        sc = ppC.tile([P, 1024], f32, tag="C", name="sc")
                        nc.tensor.matmul(sc[:, c0:512],
                                         kT[:, 2 * p * P:(2 * p + 1) * P],
                                         qs[:, c0:], start=True, stop=True)
                        nc.tensor.matmul(sc[:, 512 + c1:],
                                         kT[:, (2 * p + 1) * P:(2 * p + 2) * P],
                                         qs[:, c1:], start=True, stop=True)
                        ep = pep.tile([P, 1024], bf16, tag="ep")
                        nc.scalar.activation(ep[:], sc[:], AF.Exp)
                        if p >= 2 * g:
                            nc.vector.tensor_tensor(
                                ep[:], ep[:], mask_sb[:, p - 2 * g, :],
                                AL.mult)
                        es = pes.tile([P, 512], bf16, tag="es")
                        nc.vector.tensor_tensor(es[:], ep[:, :512],
                                                ep[:, 512:], AL.add)
                        if p % 2 == 1:
                            # fold two pair-sums; accumulate the denominator
                            # partial sums in f32 on DVE (no PE involvement)
                            e2 = pe2.tile([P, 512], bf16, tag="es2")
                            nc.vector.tensor_tensor(
                                e2[:], ess.pop(p - 1)[:], es[:], AL.add)
                            if acc is None:
                                acc = e2
                            else:
                                nacc = pacc.tile([P, 512], f32, tag="acc")
                                nc.vector.tensor_tensor(
                                    nacc[:], acc[:], e2[:], AL.add)
                                acc = nacc
                        else:
                            ess[p] = es

                        def consume(ep=ep, p=p, g=g, ya=ya,
                                    nj=nj, npair=npair):
                            for half in range(2):
                                jt = 2 * p + half
                                eph = ep[:, half * 512:(half + 1) * 512]
                                dt = jt - 4 * g
                                if dt < 0:
                                    nc.tensor.matmul(ya[:], vN[:, jt, :],
                                                     eph,
                                                     start=(jt == 0),
                                                     stop=False)
                                else:
                                    # column block dt takes its last
                                    # contribution here; later blocks go on
                                    b0, b1 = dt * P, (dt + 1) * P
                                    nc.tensor.matmul(ya[:, b0:b1],
                                                     vN[:, jt, :],
                                                     eph[:, b0:b1],
                                                     start=(jt == 0),
                                                     stop=True)
                                    if dt < 3:
                                        nc.tensor.matmul(ya[:, b1:512],
                                                         vN[:, jt, :],
                                                         eph[:, b1:512],
                                                         start=(jt == 0),
                                                         stop=False)
                        pending.append(consume)
                        pump(4)
                    pending.append(fin(h, ya, acc, yt))
                    if g == 1 and h in (0, 1):
                        # late qk transposes, covered by attention work
                        pending.append(lambda t=14 + h: emit_tp(t))
                items = oproj(g, yt)
                pending.append(lambda items=items: fillers.extend(items))
            pump(0)

        while fillers:
            fillers.popleft()()

    nc.compile()
    return nc


def _get_program(loop_n=0):
    key = loop_n
    if key not in _PROG:
        _PROG[key] = _build_program(loop_n)
    return _PROG[key]


def _host_prep(x, Wq, Wk, Wv, Wp, q_gain):
    """Build the 8 per-core input maps. Core c = (batch c//4, kv head c%4)."""
    import ml_dtypes
    bf16 = ml_dtypes.bfloat16

    inv_freq = 1.0 / (ROPE_BASE ** (np.arange(0, HD, 2, dtype=np.float32) / HD))
    freqs = np.arange(S, dtype=np.float32)[:, None] * inv_freq[None, :]
    cos = np.ascontiguousarray(np.cos(freqs).astype(np.float32))
    sin = np.ascontiguousarray(np.sin(freqs).astype(np.float32))

    # causal 0/1 masks for the diagonal-chunk tile variants (il = 0..3),
    # packed as pairs: variant v holds [il=2v | il=2v+1] side by side.
    # tile rows k (128), group columns q (512): valid iff q >= il*128 + k
    k = np.arange(P)[:, None, None]
    il = np.arange(4)[None, :, None]
    q = np.arange(512)[None, None, :]
    masks = (q >= il * P + k).astype(bf16)               # [128, 4, 512]
    masks = masks.reshape(P, 2, 1024)                    # [128, 2, 1024]

    in_maps = []
    for core in range(N_CORES):
        b, kv = divmod(core, 4)
        h0 = 4 * kv
        xT = np.ascontiguousarray(
            x[b].reshape(S, D).T.astype(bf16))           # [D, S]
        WqT = Wq[h0 * HD:(h0 + 4) * HD, :].T             # [D, 512]
        WkT = Wk[kv * HD:(kv + 1) * HD, :].T             # [D, 128]
        WvT = Wv[kv * HD:(kv + 1) * HD, :].T             # [D, 128]
        wcat = np.ascontiguousarray(
            np.concatenate([WqT, WkT, WvT], axis=1).astype(bf16))
        wpT = np.ascontiguousarray(
            Wp[:, h0 * HD:(h0 + 4) * HD].T.astype(bf16))  # [512, D]
        gain = np.tile(np.array(
            [[q_gain[h0] * SCALE, q_gain[h0 + 1] * SCALE,
              q_gain[h0 + 2] * SCALE, q_gain[h0 + 3] * SCALE,
              1.0, EPS]], dtype=np.float32), (P, 1))
        in_maps.append({
            "xT": xT,
            "wcat": wcat,
            "wpd": wpT,
            "cosd": cos,
            "sind": sin,
            "maskd": np.ascontiguousarray(masks),
            "gaind": np.ascontiguousarray(gain),
        })
    return in_maps


def kernel(x, Wq, Wk, Wv, Wp, q_gain):
    from concourse.bass_utils import run_bass_kernel_spmd

    nc = _get_program()
    in_maps = _host_prep(x, Wq, Wk, Wv, Wp, q_gain)
    try:
        res = run_bass_kernel_spmd(nc, in_maps, core_ids=list(range(N_CORES)))
    except Exception:
        # one retry: a previous crashed run can leave the exec unit wedged
        res = run_bass_kernel_spmd(nc, in_maps, core_ids=list(range(N_CORES)))
    out = np.zeros((B, S, D), dtype=np.float32)
    for core in range(N_CORES):
        out[core // 4] += res.results[core]["out"]
    return out
